# revision 1
# baseline (speedup 1.0000x reference)
"""BRD4KANModel Trainium2 kernel.

Data-parallel over batch across 8 NeuronCores (512 rows each, weights
replicated). On-chip layout is feature-major (h^T: features on partitions,
batch on the free dim), so every layer's matmul output [out_feat, batch]
feeds the next layer directly. Weights arrive (out, in) row-major; the PE
needs the contraction dim on partitions, so each 128x128 weight tile is
cast to bf16 during the SWDGE DMA and transposed on-chip with a PE
transpose (is_transpose matmul vs identity), evacuated PSUM->SBUF by the
scalar engine.

B-spline bases use the truncated-power form: with z_m = lam*relu(x - g_m),
lam = (6h^3)^(-1/3), the 6 cubic bases are the 4th forward differences of
z_m^3 — identical to the Cox-de Boor reference (and exactly 0 outside the
grid) up to fp32 cancellation ~1e-3 absolute.

This walrus build accepts only ONE semaphore wait per instruction, while
Tile's scheduler attaches several; _split_waits() post-processes the BIR
JSON, hoisting excess waits onto NoOps inserted just before each
instruction on the same engine.
"""

import json
import os

import numpy as np

import concourse.bass as bass
import concourse.mybir as mybir
import concourse.tile as tile
from concourse.masks import make_identity

F32 = mybir.dt.float32
BF16 = mybir.dt.bfloat16
AF = mybir.ActivationFunctionType
OP = mybir.AluOpType

N_CORES = 8
BATCH = 4096
B = BATCH // N_CORES  # 512 per core
D = 2048
WIDTHS = [2048, 2048, 1024]
COEFF = 6
GRID_SIZE = 3
SPLINE_ORDER = 3
H = 2.0 / GRID_SIZE
GRID = [m * H - 1.0 - SPLINE_ORDER * H for m in range(GRID_SIZE + 2 * SPLINE_ORDER + 1)]
LAM = float((6.0 * H**3) ** (-1.0 / 3.0))

CH = 256          # spline i-chunk (features per weight DMA chunk)
ZW = 256          # bases compute width (sub-batch per DVE pass)


def _split_waits(bir_bytes: bytes, keep: int = 1) -> bytes:
    d = json.loads(bir_bytes)
    for f in d["functions"]:
        for bb in f["blocks"]:
            new_insts = []
            for inst in bb["instructions"]:
                si = inst.get("sync_info")
                waits = (si or {}).get("on_wait") or []
                if len(waits) > keep:
                    extra = waits[:-keep]
                    inst["sync_info"]["on_wait"] = waits[-keep:]
                    for ci in range(0, len(extra), keep):
                        new_insts.append({
                            "name": f"{inst['name']}-w{ci}",
                            "opcode": "NoOp",
                            "engine": inst["engine"],
                            "ins": [],
                            "outs": [],
                            "debug": inst.get("debug"),
                            "sync_info": {"on_update": [],
                                          "on_wait": extra[ci:ci + keep]},
                        })
                new_insts.append(inst)
            bb["instructions"] = new_insts
    return json.dumps(d).encode()


def _patch_json(nc):
    orig = nc.to_json_bytes

    def patched():
        return _split_waits(orig())

    nc.to_json_bytes = patched
    return nc


def build(stage=99):
    nc = bass.Bass()
    x = nc.dram_tensor("x", [B, D], F32, kind="ExternalInput")
    mult_w = nc.dram_tensor("mult_w", [2 * D, D], F32, kind="ExternalInput")
    mult_b = nc.dram_tensor("mult_b", [2 * D], F32, kind="ExternalInput")
    kan = []
    dims = [D] + WIDTHS
    for l in range(3):
        fo = dims[l + 1]
        kan.append((
            nc.dram_tensor(f"base_w{l}", [fo, dims[l]], F32, kind="ExternalInput"),
            nc.dram_tensor(f"spline_w{l}", [fo, dims[l], COEFF], F32, kind="ExternalInput"),
            nc.dram_tensor(f"scaler{l}", [fo, dims[l]], F32, kind="ExternalInput"),
        ))
    reg_w = nc.dram_tensor("reg_w", [1, WIDTHS[-1]], F32, kind="ExternalInput")
    reg_b = nc.dram_tensor("reg_b", [1], F32, kind="ExternalInput")
    aux_w = nc.dram_tensor("aux_w", [1, WIDTHS[-1]], F32, kind="ExternalInput")
    aux_b = nc.dram_tensor("aux_b", [1], F32, kind="ExternalInput")
    out = nc.dram_tensor("out", [2, B], F32, kind="ExternalOutput")
    dbg = nc.dram_tensor("dbg", [128, B], F32, kind="ExternalOutput")

    with tile.TileContext(nc) as tc:
        with tc.tile_pool(name="consts", bufs=1) as consts, \
             tc.tile_pool(name="hp", bufs=20) as hp, \
             tc.tile_pool(name="rhs", bufs=16) as rhsp, \
             tc.tile_pool(name="bases", bufs=96) as basesp, \
             tc.tile_pool(name="zp", bufs=11) as zp, \
             tc.tile_pool(name="z2p", bufs=2) as z2p, \
             tc.tile_pool(name="wload", bufs=2) as wload, \
             tc.tile_pool(name="scload", bufs=2) as scload, \
             tc.tile_pool(name="sw", bufs=2) as swp, \
             tc.tile_pool(name="wT", bufs=8) as wTp, \
             tc.tile_pool(name="h2", bufs=8) as h2p, \
             tc.tile_pool(name="psA", bufs=4, space="PSUM") as psA, \
             tc.tile_pool(name="psT", bufs=4, space="PSUM") as psT:

            ident = consts.tile([128, 128], BF16, tag="ident")
            make_identity(nc, ident)
            mb_sb = consts.tile([128, 32], F32, tag="mb")
            nc.sync.dma_start(mb_sb, mult_b[:].rearrange("(t p) -> p t", p=128))
            hw_sb = consts.tile([2, 1024], BF16, tag="hw")
            nc.gpsimd.dma_start(hw_sb[0:1, :], reg_w[:])
            nc.gpsimd.dma_start(hw_sb[1:2, :], aux_w[:])
            hb_sb = consts.tile([2, 1], F32, tag="hb")
            nc.sync.dma_start(hb_sb[0:1, :], reg_b[None, :])
            nc.sync.dma_start(hb_sb[1:2, :], aux_b[None, :])
            grid_sb = consts.tile([128, 10], F32, tag="grid")
            for m in range(10):
                nc.vector.memset(grid_sb[:, m:m + 1], float(-LAM * GRID[m]))

            def transpose_tile(src_ap):
                """src [128,128] bf16 (any strided slice) -> transposed SBUF bf16."""
                pt = psT.tile([128, 128], BF16, tag="pt", name="pt1")
                nc.tensor.transpose(pt, src_ap, ident)
                dst = wTp.tile([128, 512], BF16, tag="wT", name="wT1")
                nc.scalar.copy(dst[:, 0:128], pt)
                return dst[:, 0:128]

            tg_n = [0]

            def transpose_group(srcs):
                """<=4 [128,128] bf16 APs -> one PSUM bank -> one batched evac."""
                n = len(srcs)
                pt = psT.tile([128, 512], BF16, tag="pt", name="ptg")
                for q, sap in enumerate(srcs):
                    nc.tensor.transpose(pt[:, q * 128:(q + 1) * 128], sap, ident)
                wt = wTp.tile([128, 512], BF16, tag="wT", name="wTg")
                nc.scalar.copy(wt[:, :n * 128], pt[:, :n * 128])
                return [wt[:, q * 128:(q + 1) * 128] for q in range(n)]

            # ---- x^T: cast x to bf16 and PE-transpose into feature-major ----
            IT0 = D // 128  # 16
            xb = []  # xb[i] [128, B] bf16, partitions = features
            for i in range(IT0):
                xb.append(rhsp.tile([128, B], BF16, tag="rhs", name=f"xb{i}"))
            for bt in range(B // 128):  # 4 batch tiles
                xf = wload.tile([128, D], BF16, tag="wload")
                nc.gpsimd.dma_start(xf, x[bt * 128:(bt + 1) * 128, :])
                for i in range(IT0):
                    pt = psT.tile([128, 128], BF16, tag="pt")
                    nc.tensor.transpose(pt, xf[:, i * 128:(i + 1) * 128], ident)
                    nc.scalar.copy(xb[i][:, bt * 128:(bt + 1) * 128], pt)

            # ---- multiplicative layer: hh = x @ mult_w.T + b; h=sig(gate)*val
            h_tiles = []
            for j in range(IT0):  # output tiles of h (2048 feats)
                sig = None
                for half, o in ((0, j), (1, j + 16)):
                    acc = psA.tile([128, B], F32, tag="acc")
                    wstrip = wload.tile([128, D], BF16, tag="wload")
                    nc.gpsimd.dma_start(wstrip, mult_w[o * 128:(o + 1) * 128, :])
                    for i0 in range(0, IT0, 4):
                        wTs = transpose_group(
                            [wstrip[:, i * 128:(i + 1) * 128]
                             for i in range(i0, i0 + 4)])
                        for q, i in enumerate(range(i0, i0 + 4)):
                            nc.tensor.matmul(acc, wTs[q], xb[i],
                                             start=(i == 0),
                                             stop=(i == IT0 - 1))
                    if half == 0:
                        sig = hp.tile([128, B], F32, tag="h")
                        nc.scalar.activation(sig, acc, AF.Sigmoid,
                                             bias=mb_sb[:, j:j + 1])
                    else:
                        val = hp.tile([128, B], F32, tag="h")
                        nc.vector.tensor_scalar(val, acc, mb_sb[:, 16 + j:17 + j],
                                                None, OP.add)
                        ht = hp.tile([128, B], F32, tag="h")
                        nc.vector.tensor_tensor(ht, sig, val, OP.mult)
                        h_tiles.append(ht)

            # ---- KAN layers: 2-sweep k-split pipeline ----
            # B-V1(l) [k-half 0] overlaps A2(l) [bases i-half 1];
            # B-V2(l) overlaps A1(l+1). SBUF f32 accumulator carries V1->V2.
            silu_t = {}
            bas_t = {}

            def phase_a_half(l, hf, h_in):
                """silu + bases for i in [hf*IT/2, (hf+1)*IT/2) of layer l."""
                fi = dims[l]
                IT = fi // 128
                for i in range(hf * IT // 2, (hf + 1) * IT // 2):
                    st = rhsp.tile([128, B], BF16, tag="rhs", name=f"silu{l}_{i}")
                    nc.scalar.activation(st, h_in[i], AF.Silu)
                    silu_t[(l, i)] = st
                    bt6 = [basesp.tile([128, B], BF16, tag="bases",
                                       name=f"bas{l}_{i}_{c}")
                           for c in range(COEFF)]
                    bas_t[(l, i)] = bt6
                    for w0 in range(0, B, ZW):
                        sl = slice(w0, w0 + ZW)
                        z = []
                        z2 = z2p.tile([128, ZW], F32, tag="z2")
                        for m in range(10):
                            zm = zp.tile([128, ZW], F32, tag="z", name=f"z{m}")
                            nc.scalar.activation(zm, h_in[i][:, sl], AF.Relu,
                                                 bias=grid_sb[:, m:m + 1],
                                                 scale=LAM)
                            nc.scalar.square(z2, zm)
                            nc.vector.tensor_tensor(zm, z2, zm, OP.mult)
                            z.append(zm)
                        for r in range(3):
                            for m in range(9 - r):
                                nc.vector.tensor_tensor(z[m], z[m], z[m + 1],
                                                        OP.subtract)
                        for c in range(COEFF):
                            nc.vector.tensor_tensor(bt6[c][:, sl], z[c],
                                                    z[c + 1], OP.subtract)

            def phase_b_half(l, hf, hacc):
                """matmul sweep over k-half hf; accumulate into hacc tiles."""
                bw_d, sw_d, sc_d = kan[l]
                fi, fo = dims[l], dims[l + 1]
                IT, OT = fi // 128, fo // 128
                last = (l == 2)
                i_lo, i_hi = hf * IT // 2, (hf + 1) * IT // 2
                K_V = (i_hi - i_lo) * 7
                out_tiles = []
                for o in range(OT):
                    acc = psA.tile([128, B], F32, tag="acc")
                    osl = slice(o * 128, (o + 1) * 128)
                    bstrip = wload.tile([128, fi // 2], BF16, tag="wload")
                    nc.gpsimd.dma_start(bstrip, bw_d[osl, i_lo * 128:i_hi * 128])
                    k = 0
                    for i0 in range(i_lo, i_hi, 4):
                        wTs = transpose_group(
                            [bstrip[:, (i - i_lo) * 128:(i - i_lo + 1) * 128]
                             for i in range(i0, i0 + 4)])
                        for q, i in enumerate(range(i0, i0 + 4)):
                            nc.tensor.matmul(acc, wTs[q], silu_t[(l, i)],
                                             start=(k == 0),
                                             stop=(k == K_V - 1))
                            k += 1
                    for ic in range(i_lo * 128 // CH, i_hi * 128 // CH):
                        swt = swp.tile([128, CH * COEFF], BF16, tag="sw")
                        nc.gpsimd.dma_start(
                            swt, sw_d[osl, ic * CH:(ic + 1) * CH, :]
                            .rearrange("o i c -> o (i c)"))
                        sct = scload.tile([128, CH], BF16, tag="sc")
                        nc.gpsimd.dma_start(sct, sc_d[osl, ic * CH:(ic + 1) * CH])
                        sw3 = swt.rearrange("p (i c) -> p i c", c=COEFF)
                        nc.vector.tensor_tensor(
                            sw3, sw3,
                            sct[:, :, None].to_broadcast(sw3.shape), OP.mult)
                        views = []
                        for isub in range(CH // 128):
                            i_g = ic * (CH // 128) + isub
                            for c in range(COEFF):
                                views.append(
                                    (sw3[:, isub * 128:(isub + 1) * 128, c],
                                     bas_t[(l, i_g)][c]))
                        for v0 in range(0, len(views), 4):
                            grp = views[v0:v0 + 4]
                            wTs = transpose_group([g[0] for g in grp])
                            for q, (_, rhs_t) in enumerate(grp):
                                nc.tensor.matmul(acc, wTs[q], rhs_t,
                                                 start=(k == 0),
                                                 stop=(k == K_V - 1))
                                k += 1
                    if hf == 0:
                        ht = hp.tile([128, B], F32, tag="h", name=f"hacc{l}_{o}")
                        nc.scalar.copy(ht, acc)
                        out_tiles.append(ht)
                    else:
                        if last:
                            ot = h2p.tile([128, B], BF16, tag="h2",
                                          name=f"h2_{o}")
                            nc.vector.tensor_tensor(ot, acc, hacc[o], OP.add)
                            out_tiles.append(ot)
                        else:
                            nc.vector.tensor_tensor(hacc[o], acc, hacc[o],
                                                    OP.add)
                            out_tiles.append(hacc[o])
                return out_tiles

            n_layers = min(3, max(0, stage - 1))
            if n_layers:
                phase_a_half(0, 0, h_tiles)
            cur_h = h_tiles
            for l in range(n_layers):
                phase_a_half(l, 1, cur_h)
                hacc = phase_b_half(l, 0, None)
                if l + 1 < n_layers:
                    # A1(l+1) emitted after V2(l) below needs V2 outputs;
                    # emit V2 first, then A1(l+1) (scheduler overlaps them).
                    pass
                new_h = phase_b_half(l, 1, hacc)
                cur_h = new_h
                if l + 1 < n_layers:
                    phase_a_half(l + 1, 0, cur_h)
            h_tiles = cur_h

            # ---- debug tap: first live tile of h_tiles ----
            if stage < 5:
                dbg_t = hp.tile([128, B], F32, tag="h", name="dbgt")
                nc.vector.tensor_copy(dbg_t, h_tiles[0])
                nc.sync.dma_start(dbg[:], dbg_t)

            # ---- heads ----
            if stage >= 5:
                acc = psA.tile([128, B], F32, tag="acc")
                IT2 = WIDTHS[-1] // 128  # 8
                for i in range(IT2):
                    pt = psT.tile([128, 128], BF16, tag="pt")
                    nc.tensor.transpose(pt[:, 0:2],
                                        hw_sb[:, i * 128:(i + 1) * 128],
                                        ident[0:2, 0:2])
                    wT = wTp.tile([128, 128], BF16, tag="wT")
                    nc.scalar.copy(wT[:, 0:2], pt[:, 0:2])
                    nc.tensor.matmul(acc[0:2, :], wT[:, 0:2], h_tiles[i],
                                     start=(i == 0), stop=(i == IT2 - 1))
                res = consts.tile([2, B], F32, tag="res")
                nc.vector.tensor_scalar(res, acc[0:2, :], hb_sb[:, 0:1], None,
                                        OP.add)
                nc.sync.dma_start(out[:], res)

    return _patch_json(nc)


_NC = None


def kernel(**inputs):
    global _NC
    from concourse.bass_utils import run_bass_kernel_spmd

    if _NC is None:
        _NC = build(int(os.environ.get("KSTAGE", "99")))
    per_core = []
    x_full = np.ascontiguousarray(inputs["x"], dtype=np.float32)
    shared = {k: np.ascontiguousarray(np.asarray(v), dtype=np.float32)
              for k, v in inputs.items() if k != "x"}
    for c in range(N_CORES):
        m = dict(shared)
        m["x"] = np.ascontiguousarray(x_full[c * B:(c + 1) * B])
        per_core.append(m)
    res = run_bass_kernel_spmd(_NC, per_core, core_ids=list(range(N_CORES)))
    reg = np.concatenate([res.results[c]["out"][0] for c in range(N_CORES)])
    aux = np.concatenate([res.results[c]["out"][1] for c in range(N_CORES)])
    kernel.last_results = res
    return reg, aux



# revision 7
# speedup vs baseline: 1.5634x; 1.5634x over previous
"""BRD4KANModel Trainium2 kernel (v2).

Data-parallel over batch across 8 NeuronCores (512 rows each, weights
replicated). All weights are pre-transposed / pre-tiled / bf16-cast on the
host into matmul-ready lhsT layout ([in-feature partitions, out-feature
free]), with the spline scaler and the truncated-power scale lam folded in.
The device therefore runs ONLY real matmuls on the PE (no transposes), the
B-spline bases on ACT+DVE(+POOL), and PSUM evacuations.

B-spline bases via truncated powers: with h' = lam*h and c_m = lam*g_m,
z_m = relu(h' - c_m), the 6 cubic bases are the 4th forward differences
b_c = z³_c - 4z³_{c+1} + 6z³_{c+2} - 4z³_{c+3} + z³_{c+4}, computed as a
grouped 24-op DVE cascade per 128-feature tile. z² comes from one ACT
Square (bias = -c_m) and z³ = relu * z² runs on POOL (or DVE via env
BASS_CUBE=dve).

Layer matmuls are split into 4 k-quarters (i-tiles 0-3, 4-7, 8-11, 12-15)
so only ~2 quarters of bases tiles are ever live (SBUF), with an f32 SBUF
accumulator carrying partial sums between quarters. Bases for the next
consumer sweep are emitted one sweep ahead, overlapping DVE/ACT/POOL work
with PE matmuls.

This walrus build accepts only ONE semaphore wait per instruction, while
Tile's scheduler attaches several; _split_waits() post-processes the BIR
JSON, hoisting excess waits onto NoOps inserted just before each
instruction on the same engine.
"""

import json
import os

import numpy as np
import ml_dtypes

import concourse.bass as bass
import concourse.mybir as mybir
import concourse.tile as tile

F32 = mybir.dt.float32
BF16 = mybir.dt.bfloat16
AF = mybir.ActivationFunctionType
OP = mybir.AluOpType

N_CORES = 8
BATCH = 4096
B = BATCH // N_CORES  # 512 per core
D = 2048
WIDTHS = [2048, 2048, 1024]
COEFF = 6
GRID_SIZE = 3
SPLINE_ORDER = 3
HSTEP = 2.0 / GRID_SIZE
GRID = [m * HSTEP - 1.0 - SPLINE_ORDER * HSTEP
        for m in range(GRID_SIZE + 2 * SPLINE_ORDER + 1)]  # 10 knots, -3..3
LAM = float((6.0 * HSTEP ** 3) ** (-1.0 / 3.0))
NK = 10          # truncated-power knots
IT = 16          # 2048/128 input tiles per layer
NQ = 4           # k-quarters
KQ = IT // NQ    # i-tiles per quarter
SW = 512 + KQ * COEFF * 128  # combined strip width per (o,q): base + spline


def _split_waits(bir_bytes: bytes, keep: int = 1) -> bytes:
    d = json.loads(bir_bytes)
    for f in d["functions"]:
        for bb in f["blocks"]:
            new_insts = []
            for inst in bb["instructions"]:
                si = inst.get("sync_info")
                waits = (si or {}).get("on_wait") or []
                if len(waits) > keep:
                    extra = waits[:-keep]
                    inst["sync_info"]["on_wait"] = waits[-keep:]
                    for ci in range(0, len(extra), keep):
                        new_insts.append({
                            "name": f"{inst['name']}-w{ci}",
                            "opcode": "NoOp",
                            "engine": inst["engine"],
                            "ins": [],
                            "outs": [],
                            "debug": inst.get("debug"),
                            "sync_info": {"on_update": [],
                                          "on_wait": extra[ci:ci + keep]},
                        })
                new_insts.append(inst)
            bb["instructions"] = new_insts
    return json.dumps(d).encode()


def _patch_json(nc):
    orig = nc.to_json_bytes

    def patched():
        return _split_waits(orig())

    nc.to_json_bytes = patched
    return nc


def build():
    cube_eng = os.environ.get("BASS_CUBE", "pool")
    nc = bass.Bass()
    xT = nc.dram_tensor("xT", [D, B], BF16, kind="ExternalInput")
    wm = nc.dram_tensor("wm", [32 * 128, D], BF16, kind="ExternalInput")
    mbg = nc.dram_tensor("mbg", [128, 16], F32, kind="ExternalInput")
    mbv = nc.dram_tensor("mbv", [128, 16], F32, kind="ExternalInput")
    ws_d = []
    for l, fo in enumerate(WIDTHS):
        ot = fo // 128
        ws_d.append(nc.dram_tensor(f"ws{l}", [ot * NQ * 128, SW], BF16,
                                   kind="ExternalInput"))
    wh = nc.dram_tensor("wh", [128, 16], BF16, kind="ExternalInput")
    hb = nc.dram_tensor("hb", [2, 1], F32, kind="ExternalInput")
    out = nc.dram_tensor("out", [2, B], F32, kind="ExternalOutput")

    with tile.TileContext(nc) as tc:
        with tc.tile_pool(name="consts", bufs=1) as consts, \
             tc.tile_pool(name="wmp", bufs=2) as wmp, \
             tc.tile_pool(name="wsp", bufs=2) as wsp, \
             tc.tile_pool(name="hp", bufs=38) as hp, \
             tc.tile_pool(name="silup", bufs=12) as silup, \
             tc.tile_pool(name="basp", bufs=49) as basp, \
             tc.tile_pool(name="zp", bufs=10) as zp, \
             tc.tile_pool(name="rtp", bufs=4) as rtp, \
             tc.tile_pool(name="qp", bufs=5) as qp, \
             tc.tile_pool(name="h3p", bufs=8) as h3p, \
             tc.tile_pool(name="psA", bufs=6, space="PSUM") as psA, \
             tc.tile_pool(name="psH", bufs=1, space="PSUM") as psH:

            # ---- constants ----
            cm = consts.tile([128, NK], F32, tag="cm")
            for m in range(NK):
                nc.vector.memset(cm[:, m:m + 1], float(-LAM * GRID[m]))
            mbg_sb = consts.tile([128, 16], F32, tag="mbg")
            nc.sync.dma_start(mbg_sb, mbg[:])
            mbv_sb = consts.tile([128, 16], F32, tag="mbv")
            nc.sync.dma_start(mbv_sb, mbv[:])
            wh_sb = consts.tile([128, 16], BF16, tag="wh")
            nc.sync.dma_start(wh_sb, wh[:])
            hb_sb = consts.tile([2, 1], F32, tag="hb")
            nc.sync.dma_start(hb_sb, hb[:])

            # ---- x^T tiles (host pre-transposed; share bas slots) ----
            xb = []
            for i in range(IT):
                t = basp.tile([128, B], BF16, tag="bas", name=f"x{i}")
                nc.sync.dma_start(t, xT[i * 128:(i + 1) * 128, :])
                xb.append(t)

            silu_t = {}
            bas_t = {}

            def emit_A(l, i, h_t):
                """silu + 6 b-spline bases tiles for layer-l input tile i."""
                st = silup.tile([128, B], BF16, tag="silu",
                                name=f"silu{l}_{i}")
                if l == 0:
                    # keep the ACT stream on the sigmoid table-set while the
                    # multiplicative layer's sigmoids are interleaved
                    sg = qp.tile([128, B], F32, tag="q", name=f"sg{i}")
                    nc.scalar.activation(sg, h_t, AF.Sigmoid,
                                         scale=1.0 / LAM)
                    nc.vector.scalar_tensor_tensor(st, h_t, 1.0 / LAM, sg,
                                                   OP.mult, OP.mult)
                else:
                    nc.scalar.activation(st, h_t, AF.Silu, scale=1.0 / LAM)
                silu_t[(l, i)] = st
                z3 = []
                for m in range(NK):
                    r = rtp.tile([128, B], F32, tag="rt", name=f"r{m}")
                    nc.scalar.activation(r, h_t, AF.Relu,
                                         bias=cm[:, m:m + 1])
                    t2 = rtp.tile([128, B], F32, tag="rt", name=f"t2{m}")
                    nc.scalar.activation(t2, h_t, AF.Square,
                                         bias=cm[:, m:m + 1])
                    z = zp.tile([128, B], F32, tag="z", name=f"z3_{m}")
                    if cube_eng == "pool":
                        nc.gpsimd.tensor_tensor(z, r, t2, OP.mult)
                    else:
                        nc.vector.tensor_tensor(z, r, t2, OP.mult)
                    z3.append(z)
                for c in range(COEFF):
                    q = qp.tile([128, B], F32, tag="q", name=f"q{c}")
                    nc.vector.tensor_tensor(q, z3[c], z3[c + 4], OP.add)
                    r2 = qp.tile([128, B], F32, tag="q", name=f"r2{c}")
                    nc.vector.tensor_tensor(r2, z3[c + 1], z3[c + 3], OP.add)
                    nc.vector.scalar_tensor_tensor(q, r2, -4.0, q,
                                                   OP.mult, OP.add)
                    bt = basp.tile([128, B], BF16, tag="bas",
                                   name=f"bas{l}_{i}_{c}")
                    nc.vector.scalar_tensor_tensor(bt, z3[c + 2], 6.0, q,
                                                   OP.mult, OP.add)
                    bas_t[(l, i, c)] = bt

            # ---- multiplicative layer ----
            h_cur = []
            for j in range(IT):
                wg = wmp.tile([128, D], BF16, tag="wm", name=f"wg{j}")
                nc.sync.dma_start(wg, wm[j * 128:(j + 1) * 128, :])
                accg = psA.tile([128, B], F32, tag="acc")
                for k in range(IT):
                    nc.tensor.matmul(accg, wg[:, k * 128:(k + 1) * 128],
                                     xb[k], start=(k == 0),
                                     stop=(k == IT - 1))
                sig = qp.tile([128, B], F32, tag="q", name=f"sig{j}")
                nc.scalar.activation(sig, accg, AF.Sigmoid,
                                     bias=mbg_sb[:, j:j + 1])
                wv = wmp.tile([128, D], BF16, tag="wm", name=f"wv{j}")
                nc.sync.dma_start(wv, wm[(16 + j) * 128:(17 + j) * 128, :])
                accv = psA.tile([128, B], F32, tag="acc")
                for k in range(IT):
                    nc.tensor.matmul(accv, wv[:, k * 128:(k + 1) * 128],
                                     xb[k], start=(k == 0),
                                     stop=(k == IT - 1))
                ht = hp.tile([128, B], F32, tag="h", name=f"h0_{j}")
                nc.vector.scalar_tensor_tensor(ht, accv, mbv_sb[:, j:j + 1],
                                               sig, OP.add, OP.mult)
                h_cur.append(ht)
                # only quarter 0 here: more would exhaust bas slots (shared
                # with x tiles) and wedge the DVE queue behind slot waits
                if j < KQ:
                    emit_A(0, j, ht)

            # ---- KAN layers: 4-quarter k-split sweeps ----
            h3 = []
            for l in range(3):
                ot = WIDTHS[l] // 128
                hacc = [None] * ot
                for q in range(NQ):
                    for o in range(ot):
                        strip = wsp.tile([128, SW], BF16, tag="ws",
                                         name=f"ws{l}_{q}_{o}")
                        row = (o * NQ + q) * 128
                        nc.sync.dma_start(strip, ws_d[l][row:row + 128, :])
                        acc = psA.tile([128, B], F32, tag="acc")
                        idx = 0
                        last = KQ * (1 + COEFF) - 1
                        for kk in range(KQ):
                            i = q * KQ + kk
                            nc.tensor.matmul(
                                acc, strip[:, kk * 128:(kk + 1) * 128],
                                silu_t[(l, i)], start=(idx == 0),
                                stop=(idx == last))
                            idx += 1
                            for c in range(COEFF):
                                o0 = 512 + (kk * COEFF + c) * 128
                                nc.tensor.matmul(
                                    acc, strip[:, o0:o0 + 128],
                                    bas_t[(l, i, c)], start=False,
                                    stop=(idx == last))
                                idx += 1
                        if q == 0:
                            hacc[o] = hp.tile([128, B], F32, tag="h",
                                              name=f"hacc{l}_{o}")
                            nc.scalar.copy(hacc[o], acc)
                        elif q < NQ - 1 or l < 2:
                            nc.vector.tensor_tensor(hacc[o], acc, hacc[o],
                                                    OP.add)
                        else:
                            h3t = h3p.tile([128, B], BF16, tag="h3",
                                           name=f"h3_{o}")
                            nc.vector.tensor_tensor(h3t, acc, hacc[o],
                                                    OP.add)
                            h3.append(h3t)
                        # weave bases one quarter ahead: during B(l,q) build
                        # bases for quarter q+1 (slots freed by sweep q-1)
                        if l + 1 < 3 and q == NQ - 1 and o < KQ:
                            emit_A(l + 1, o, hacc[o])
                        if q < NQ - 1 and o < KQ:
                            emit_A(l, KQ * (q + 1) + o,
                                   h_cur[KQ * (q + 1) + o])
                h_cur = hacc

            # ---- heads ----
            acch = psH.tile([128, B], F32, tag="acch")
            it2 = WIDTHS[-1] // 128
            for k in range(it2):
                nc.tensor.matmul(acch[0:2, :], wh_sb[:, 2 * k:2 * k + 2],
                                 h3[k], start=(k == 0), stop=(k == it2 - 1))
            res = consts.tile([2, B], F32, tag="res")
            nc.vector.tensor_scalar(res, acch[0:2, :], hb_sb[:, 0:1], None,
                                    OP.add)
            nc.sync.dma_start(out[:], res)

    return _patch_json(nc)


def _prep(inputs):
    """Host-side weight prep: fold scaler+lam, transpose, tile, bf16-cast."""
    f32 = np.float32
    bf16 = ml_dtypes.bfloat16
    feed = {}

    mw = np.asarray(inputs["mult_w"], f32).copy()  # [4096, 2048]
    mw[D:] *= LAM
    feed["wm"] = np.ascontiguousarray(
        mw.reshape(32, 128, IT, 128).transpose(0, 3, 2, 1)
        .reshape(32 * 128, D)).astype(bf16)
    mb = np.asarray(inputs["mult_b"], f32)
    feed["mbg"] = np.ascontiguousarray(mb[:D].reshape(16, 128).T).astype(f32)
    feed["mbv"] = np.ascontiguousarray(
        (LAM * mb[D:]).reshape(16, 128).T).astype(f32)

    for l, fo in enumerate(WIDTHS):
        sc_out = LAM if l < 2 else 1.0
        bw = np.asarray(inputs[f"base_w{l}"], f32) * sc_out
        sw = (np.asarray(inputs[f"spline_w{l}"], f32)
              * np.asarray(inputs[f"scaler{l}"], f32)[..., None] * sc_out)
        ot = fo // 128
        bwt = bw.reshape(ot, 128, IT, 128).transpose(0, 3, 2, 1)
        swt = sw.reshape(ot, 128, IT, 128, COEFF).transpose(0, 3, 2, 4, 1)
        arr = np.empty((ot, NQ, 128, SW), f32)
        arr[:, :, :, :512] = (bwt.reshape(ot, 128, NQ, KQ, 128)
                              .transpose(0, 2, 1, 3, 4)
                              .reshape(ot, NQ, 128, KQ * 128))
        arr[:, :, :, 512:] = (swt.reshape(ot, 128, NQ, KQ, COEFF, 128)
                              .transpose(0, 2, 1, 3, 4, 5)
                              .reshape(ot, NQ, 128, KQ * COEFF * 128))
        feed[f"ws{l}"] = np.ascontiguousarray(
            arr.reshape(ot * NQ * 128, SW)).astype(bf16)

    whh = np.stack([np.asarray(inputs["reg_w"], f32)[0],
                    np.asarray(inputs["aux_w"], f32)[0]], axis=1)  # [1024,2]
    feed["wh"] = np.ascontiguousarray(
        whh.reshape(8, 128, 2).transpose(1, 0, 2).reshape(128, 16)
    ).astype(bf16)
    feed["hb"] = np.array([[np.asarray(inputs["reg_b"], f32)[0]],
                           [np.asarray(inputs["aux_b"], f32)[0]]], f32)
    return feed


_NC = None


def kernel(**inputs):
    global _NC
    from concourse.bass_utils import run_bass_kernel_spmd

    if _NC is None:
        _NC = build()
    shared = _prep(inputs)
    x_full = np.asarray(inputs["x"], np.float32)
    per_core = []
    for c in range(N_CORES):
        m = dict(shared)
        m["xT"] = np.ascontiguousarray(
            x_full[c * B:(c + 1) * B].T).astype(ml_dtypes.bfloat16)
        per_core.append(m)
    res = run_bass_kernel_spmd(_NC, per_core, core_ids=list(range(N_CORES)))
    reg = np.concatenate([res.results[c]["out"][0] for c in range(N_CORES)])
    aux = np.concatenate([res.results[c]["out"][1] for c in range(N_CORES)])
    kernel.last_results = res
    return reg, aux


# revision 14
# speedup vs baseline: 1.5663x; 1.0018x over previous
"""BRD4KANModel Trainium2 kernel (v2).

Data-parallel over batch across 8 NeuronCores (512 rows each, weights
replicated). All weights are pre-transposed / pre-tiled / bf16-cast on the
host into matmul-ready lhsT layout ([in-feature partitions, out-feature
free]), with the spline scaler and the truncated-power scale lam folded in.
The device therefore runs ONLY real matmuls on the PE (no transposes), the
B-spline bases on ACT+DVE(+POOL), and PSUM evacuations.

B-spline bases via truncated powers: with h' = lam*h and c_m = lam*g_m,
z_m = relu(h' - c_m), the 6 cubic bases are the 4th forward differences
b_c = z³_c - 4z³_{c+1} + 6z³_{c+2} - 4z³_{c+3} + z³_{c+4}, computed as a
grouped 24-op DVE cascade per 128-feature tile. z² comes from one ACT
Square (bias = -c_m) and z³ = relu * z² runs on POOL (or DVE via env
BASS_CUBE=dve).

Layer matmuls are split into 4 k-quarters (i-tiles 0-3, 4-7, 8-11, 12-15)
so only ~2 quarters of bases tiles are ever live (SBUF), with an f32 SBUF
accumulator carrying partial sums between quarters. Bases for the next
consumer sweep are emitted one sweep ahead, overlapping DVE/ACT/POOL work
with PE matmuls.

This walrus build accepts only ONE semaphore wait per instruction, while
Tile's scheduler attaches several; _split_waits() post-processes the BIR
JSON, hoisting excess waits onto NoOps inserted just before each
instruction on the same engine.
"""

import json
import os

import numpy as np
import ml_dtypes

import concourse.bass as bass
import concourse.mybir as mybir
import concourse.tile as tile

F32 = mybir.dt.float32
BF16 = mybir.dt.bfloat16
AF = mybir.ActivationFunctionType
OP = mybir.AluOpType

N_CORES = 8
BATCH = 4096
B = BATCH // N_CORES  # 512 per core
D = 2048
WIDTHS = [2048, 2048, 1024]
COEFF = 6
GRID_SIZE = 3
SPLINE_ORDER = 3
HSTEP = 2.0 / GRID_SIZE
GRID = [m * HSTEP - 1.0 - SPLINE_ORDER * HSTEP
        for m in range(GRID_SIZE + 2 * SPLINE_ORDER + 1)]  # 10 knots, -3..3
LAM = float((6.0 * HSTEP ** 3) ** (-1.0 / 3.0))
NK = 10          # truncated-power knots
IT = 16          # 2048/128 input tiles per layer
NQ = 4           # k-quarters
KQ = IT // NQ    # i-tiles per quarter
SW = 512 + KQ * COEFF * 128  # combined strip width per (o,q): base + spline


def _split_waits(bir_bytes: bytes, keep: int = 1) -> bytes:
    d = json.loads(bir_bytes)
    for f in d["functions"]:
        for bb in f["blocks"]:
            new_insts = []
            for inst in bb["instructions"]:
                si = inst.get("sync_info")
                waits = (si or {}).get("on_wait") or []
                if len(waits) > keep:
                    extra = waits[:-keep]
                    inst["sync_info"]["on_wait"] = waits[-keep:]
                    for ci in range(0, len(extra), keep):
                        new_insts.append({
                            "name": f"{inst['name']}-w{ci}",
                            "opcode": "NoOp",
                            "engine": inst["engine"],
                            "ins": [],
                            "outs": [],
                            "debug": inst.get("debug"),
                            "sync_info": {"on_update": [],
                                          "on_wait": extra[ci:ci + keep]},
                        })
                new_insts.append(inst)
            bb["instructions"] = new_insts
    return json.dumps(d).encode()


def _patch_json(nc):
    orig = nc.to_json_bytes

    def patched():
        return _split_waits(orig())

    nc.to_json_bytes = patched
    return nc


def build():
    nc = bass.Bass()
    xT = nc.dram_tensor("xT", [D, B], BF16, kind="ExternalInput")
    wm = nc.dram_tensor("wm", [32 * 128, D], BF16, kind="ExternalInput")
    mbg = nc.dram_tensor("mbg", [128, 16], F32, kind="ExternalInput")
    mbv = nc.dram_tensor("mbv", [128, 16], F32, kind="ExternalInput")
    ws_d = []
    for l, fo in enumerate(WIDTHS):
        ot = fo // 128
        ws_d.append(nc.dram_tensor(f"ws{l}", [ot * NQ * 128, SW], BF16,
                                   kind="ExternalInput"))
    wh = nc.dram_tensor("wh", [128, 16], BF16, kind="ExternalInput")
    hb = nc.dram_tensor("hb", [2, 1], F32, kind="ExternalInput")
    out = nc.dram_tensor("out", [2, B], F32, kind="ExternalOutput")

    with tile.TileContext(nc) as tc:
        with tc.tile_pool(name="consts", bufs=1) as consts, \
             tc.tile_pool(name="wmp", bufs=2) as wmp, \
             tc.tile_pool(name="wsp", bufs=2) as wsp, \
             tc.tile_pool(name="hp", bufs=34) as hp, \
             tc.tile_pool(name="silup", bufs=21) as silup, \
             tc.tile_pool(name="basp", bufs=49) as basp, \
             tc.tile_pool(name="zp", bufs=10) as zp, \
             tc.tile_pool(name="rtp", bufs=6) as rtp, \
             tc.tile_pool(name="qp", bufs=6) as qp, \
             tc.tile_pool(name="psA", bufs=6, space="PSUM") as psA, \
             tc.tile_pool(name="psH", bufs=1, space="PSUM") as psH:

            # ---- constants ----
            cm = consts.tile([128, NK], F32, tag="cm")
            for m in range(NK):
                nc.vector.memset(cm[:, m:m + 1], float(-LAM * GRID[m]))
            mbg_sb = consts.tile([128, 16], F32, tag="mbg")
            nc.sync.dma_start(mbg_sb, mbg[:])
            mbv_sb = consts.tile([128, 16], F32, tag="mbv")
            nc.sync.dma_start(mbv_sb, mbv[:])
            wh_sb = consts.tile([128, 16], BF16, tag="wh")
            nc.sync.dma_start(wh_sb, wh[:])
            hb_sb = consts.tile([2, 1], F32, tag="hb")
            nc.sync.dma_start(hb_sb, hb[:])

            # ---- x^T tiles (host pre-transposed; share bas slots) ----
            xb = []
            for i in range(IT):
                t = basp.tile([128, B], BF16, tag="bas", name=f"x{i}")
                nc.sync.dma_start(t, xT[i * 128:(i + 1) * 128, :])
                xb.append(t)

            silu_t = {}
            bas_t = {}

            def emit_silu(l, i, h_t):
                st = silup.tile([128, B], BF16, tag="silu",
                                name=f"silu{l}_{i}")
                if l == 0:
                    # keep the ACT stream on the sigmoid table-set while the
                    # multiplicative layer's sigmoids are interleaved
                    sg = qp.tile([128, B], F32, tag="q", name=f"sg{i}")
                    nc.scalar.activation(sg, h_t, AF.Sigmoid,
                                         scale=1.0 / LAM)
                    nc.vector.scalar_tensor_tensor(st, h_t, 1.0 / LAM, sg,
                                                   OP.mult, OP.mult)
                else:
                    nc.scalar.activation(st, h_t, AF.Silu, scale=1.0 / LAM)
                silu_t[(l, i)] = st

            def emit_A(l, i, h_t, na):
                """6 b-spline bases tiles for layer-l input tile i.

                na of the 10 z^3 cubes go through ACT (exp(3*ln(relu))),
                the rest use ACT relu+square plus a DVE multiply. Splitting
                balances the two engines (DVE and ACT+POOL share nothing;
                the GPSIMD SBUF port contention makes POOL offload a wash).
                """
                z3 = []
                for m in range(NK):
                    r = rtp.tile([128, B], F32, tag="rt", name=f"r{m}")
                    nc.scalar.activation(r, h_t, AF.Relu,
                                         bias=cm[:, m:m + 1])
                    z = zp.tile([128, B], F32, tag="z", name=f"z3_{m}")
                    if m < na:
                        lnr = rtp.tile([128, B], F32, tag="rt",
                                       name=f"lnr{m}")
                        nc.scalar.activation(lnr, r, AF.Ln)
                        nc.scalar.activation(z, lnr, AF.Exp, scale=3.0)
                    else:
                        t2 = rtp.tile([128, B], F32, tag="rt",
                                      name=f"t2{m}")
                        nc.scalar.activation(t2, h_t, AF.Square,
                                             bias=cm[:, m:m + 1])
                        nc.vector.tensor_tensor(z, r, t2, OP.mult)
                    z3.append(z)
                for c in range(COEFF):
                    q = qp.tile([128, B], F32, tag="q", name=f"q{c}")
                    nc.vector.tensor_tensor(q, z3[c], z3[c + 4], OP.add)
                    r2 = qp.tile([128, B], F32, tag="q", name=f"r2{c}")
                    nc.vector.tensor_tensor(r2, z3[c + 1], z3[c + 3], OP.add)
                    nc.vector.scalar_tensor_tensor(q, r2, -4.0, q,
                                                   OP.mult, OP.add)
                    bt = basp.tile([128, B], BF16, tag="bas",
                                   name=f"bas{l}_{i}_{c}")
                    nc.vector.scalar_tensor_tensor(bt, z3[c + 2], 6.0, q,
                                                   OP.mult, OP.add)
                    bas_t[(l, i, c)] = bt

            # ---- multiplicative layer ----
            h_cur = []
            for j in range(IT):
                wg = wmp.tile([128, D], BF16, tag="wm", name=f"wg{j}")
                nc.sync.dma_start(wg, wm[j * 128:(j + 1) * 128, :])
                accg = psA.tile([128, B], F32, tag="acc")
                for k in range(IT):
                    nc.tensor.matmul(accg, wg[:, k * 128:(k + 1) * 128],
                                     xb[k], start=(k == 0),
                                     stop=(k == IT - 1))
                sig = qp.tile([128, B], F32, tag="q", name=f"sig{j}")
                nc.scalar.activation(sig, accg, AF.Sigmoid,
                                     bias=mbg_sb[:, j:j + 1])
                wv = wmp.tile([128, D], BF16, tag="wm", name=f"wv{j}")
                nc.sync.dma_start(wv, wm[(16 + j) * 128:(17 + j) * 128, :])
                accv = psA.tile([128, B], F32, tag="acc")
                for k in range(IT):
                    nc.tensor.matmul(accv, wv[:, k * 128:(k + 1) * 128],
                                     xb[k], start=(k == 0),
                                     stop=(k == IT - 1))
                ht = hp.tile([128, B], F32, tag="h", name=f"h0_{j}")
                nc.vector.scalar_tensor_tensor(ht, accv, mbv_sb[:, j:j + 1],
                                               sig, OP.add, OP.mult)
                h_cur.append(ht)
                emit_silu(0, j, ht)
                # bases for quarter 0 only: more would exhaust bas slots
                # (shared with x tiles) and wedge the DVE queue behind slot
                # waits. DVE cubes (na=0): no ln/exp while the sigmoid
                # table-set is live.
                if j < KQ:
                    emit_A(0, j, ht, 0)

            # ---- KAN layers: 4-quarter k-split sweeps ----
            h3 = []
            for l in range(3):
                ot = WIDTHS[l] // 128
                hacc = [None] * ot
                for q in range(NQ):
                    for o in range(ot):
                        strip = wsp.tile([128, SW], BF16, tag="ws",
                                         name=f"ws{l}_{q}_{o}")
                        row = (o * NQ + q) * 128
                        nc.sync.dma_start(strip, ws_d[l][row:row + 128, :])
                        acc = psA.tile([128, B], F32, tag="acc")
                        idx = 0
                        last = KQ * (1 + COEFF) - 1
                        for kk in range(KQ):
                            i = q * KQ + kk
                            nc.tensor.matmul(
                                acc, strip[:, kk * 128:(kk + 1) * 128],
                                silu_t[(l, i)], start=(idx == 0),
                                stop=(idx == last))
                            idx += 1
                            for c in range(COEFF):
                                o0 = 512 + (kk * COEFF + c) * 128
                                nc.tensor.matmul(
                                    acc, strip[:, o0:o0 + 128],
                                    bas_t[(l, i, c)], start=False,
                                    stop=(idx == last))
                                idx += 1
                        if q == 0:
                            hacc[o] = hp.tile([128, B], F32, tag="h",
                                              name=f"hacc{l}_{o}")
                            nc.scalar.copy(hacc[o], acc)
                        elif q < NQ - 1 or l < 2:
                            nc.vector.tensor_tensor(hacc[o], acc, hacc[o],
                                                    OP.add)
                        else:
                            h3t = silup.tile([128, B], BF16, tag="silu",
                                             name=f"h3_{o}")
                            nc.vector.tensor_tensor(h3t, acc, hacc[o],
                                                    OP.add)
                            h3.append(h3t)
                        # weave bases one quarter ahead: during B(l,q) build
                        # bases for quarter q+1 (slots freed by sweep q-1).
                        # At q3, batch next layer's silus (table-set
                        # grouping) and its quarter-0 bases.
                        if l + 1 < 3 and q == NQ - 1:
                            emit_silu(l + 1, o, hacc[o])
                            if o == KQ - 1:
                                for oo in range(KQ):
                                    emit_A(l + 1, oo, hacc[oo],
                                           10 if l + 1 == 2 else 5)
                        if q < NQ - 1 and o < KQ:
                            emit_A(l, KQ * (q + 1) + o,
                                   h_cur[KQ * (q + 1) + o],
                                   10 if l == 2 else 5)
                h_cur = hacc

            # ---- heads ----
            acch = psH.tile([128, B], F32, tag="acch")
            it2 = WIDTHS[-1] // 128
            for k in range(it2):
                nc.tensor.matmul(acch[0:2, :], wh_sb[:, 2 * k:2 * k + 2],
                                 h3[k], start=(k == 0), stop=(k == it2 - 1))
            res = consts.tile([2, B], F32, tag="res")
            nc.vector.tensor_scalar(res, acch[0:2, :], hb_sb[:, 0:1], None,
                                    OP.add)
            nc.sync.dma_start(out[:], res)

    return _patch_json(nc)


def _prep(inputs):
    """Host-side weight prep: fold scaler+lam, transpose, tile, bf16-cast."""
    f32 = np.float32
    bf16 = ml_dtypes.bfloat16
    feed = {}

    mw = np.asarray(inputs["mult_w"], f32).copy()  # [4096, 2048]
    mw[D:] *= LAM
    feed["wm"] = np.ascontiguousarray(
        mw.reshape(32, 128, IT, 128).transpose(0, 3, 2, 1)
        .reshape(32 * 128, D)).astype(bf16)
    mb = np.asarray(inputs["mult_b"], f32)
    feed["mbg"] = np.ascontiguousarray(mb[:D].reshape(16, 128).T).astype(f32)
    feed["mbv"] = np.ascontiguousarray(
        (LAM * mb[D:]).reshape(16, 128).T).astype(f32)

    for l, fo in enumerate(WIDTHS):
        sc_out = LAM if l < 2 else 1.0
        bw = np.asarray(inputs[f"base_w{l}"], f32) * sc_out
        sw = (np.asarray(inputs[f"spline_w{l}"], f32)
              * np.asarray(inputs[f"scaler{l}"], f32)[..., None] * sc_out)
        ot = fo // 128
        bwt = bw.reshape(ot, 128, IT, 128).transpose(0, 3, 2, 1)
        swt = sw.reshape(ot, 128, IT, 128, COEFF).transpose(0, 3, 2, 4, 1)
        arr = np.empty((ot, NQ, 128, SW), f32)
        arr[:, :, :, :512] = (bwt.reshape(ot, 128, NQ, KQ, 128)
                              .transpose(0, 2, 1, 3, 4)
                              .reshape(ot, NQ, 128, KQ * 128))
        arr[:, :, :, 512:] = (swt.reshape(ot, 128, NQ, KQ, COEFF, 128)
                              .transpose(0, 2, 1, 3, 4, 5)
                              .reshape(ot, NQ, 128, KQ * COEFF * 128))
        feed[f"ws{l}"] = np.ascontiguousarray(
            arr.reshape(ot * NQ * 128, SW)).astype(bf16)

    whh = np.stack([np.asarray(inputs["reg_w"], f32)[0],
                    np.asarray(inputs["aux_w"], f32)[0]], axis=1)  # [1024,2]
    feed["wh"] = np.ascontiguousarray(
        whh.reshape(8, 128, 2).transpose(1, 0, 2).reshape(128, 16)
    ).astype(bf16)
    feed["hb"] = np.array([[np.asarray(inputs["reg_b"], f32)[0]],
                           [np.asarray(inputs["aux_b"], f32)[0]]], f32)
    return feed


_NC = None


def kernel(**inputs):
    global _NC
    from concourse.bass_utils import run_bass_kernel_spmd

    if _NC is None:
        _NC = build()
    shared = _prep(inputs)
    x_full = np.asarray(inputs["x"], np.float32)
    per_core = []
    for c in range(N_CORES):
        m = dict(shared)
        m["xT"] = np.ascontiguousarray(
            x_full[c * B:(c + 1) * B].T).astype(ml_dtypes.bfloat16)
        per_core.append(m)
    res = run_bass_kernel_spmd(_NC, per_core, core_ids=list(range(N_CORES)))
    reg = np.concatenate([res.results[c]["out"][0] for c in range(N_CORES)])
    aux = np.concatenate([res.results[c]["out"][1] for c in range(N_CORES)])
    kernel.last_results = res
    return reg, aux
